# revision 1
# baseline (speedup 1.0000x reference)
"""Trainium2 Bass kernel: 16-head attention with LoRA (B=2, N=2048, C=1024).

Sharding: batch x sequence rows across 8 cores (core c: batch c//4, rows
(c%4)*512). Heads stay whole per core; K/V are all-gathered over the 4-core
batch group in chunks interleaved with compute. LoRA is folded into the
weights on the host (W_eff = W + 2*B@A), softmax normalization is deferred
and batched. Everything on device is computed transposed (feature dim on
partitions); the host transposes the per-core [1024, 512] output slabs back.
"""

import os
from contextlib import ExitStack

import numpy as np
import ml_dtypes

import concourse.bass as bass
import concourse.mybir as mybir
import concourse.tile as tile
from concourse.bass_utils import run_bass_kernel_spmd

B, N, C, H, D = 2, 2048, 1024, 16, 64
R = 512          # query rows per core
KT = N // 128    # 16 seq tiles of 128
BF = mybir.dt.bfloat16
F32 = mybir.dt.float32
GROUPS = [[0, 1, 2, 3], [4, 5, 6, 7]]


def _ap(src, dims):
    """Rebuild an AP keeping its partition dim but with custom free dims."""
    return bass.AP(tensor=src.tensor, offset=src.offset,
                   ap=[list(src.ap[0])] + [list(d) for d in dims])


def build():
    nc = bass.Bass()
    xT = nc.declare_dram_parameter("xT", [C, R], BF, isOutput=False)
    wkT = nc.declare_dram_parameter("wkT", [C, C], BF, isOutput=False)
    wqT = nc.declare_dram_parameter("wqT", [C, C], BF, isOutput=False)
    wvT = nc.declare_dram_parameter("wvT", [C, C], BF, isOutput=False)
    projT = nc.declare_dram_parameter("projT", [C, C], BF, isOutput=False)
    biasT = nc.declare_dram_parameter("biasT", [1, C], BF, isOutput=False)
    outT = nc.declare_dram_parameter("outT", [C, R], F32, isOutput=True)

    with tile.TileContext(nc) as tc, ExitStack() as ctx:
        dram = ctx.enter_context(tc.tile_pool(name="dram", bufs=1, space="DRAM"))
        warm_in = dram.tile([1, 128], BF)
        warm_out = dram.tile([4, 128], BF)
        kA_b = dram.tile([4 * 128, R], BF)
        kB_b = dram.tile([4 * 128, R], BF)
        kA_g = dram.tile([4 * 4 * 128, R], BF)
        kB_g = dram.tile([4 * 4 * 128, R], BF)
        vA_b = dram.tile([R, 520], BF)
        vB_b = dram.tile([R, 520], BF)
        vA_g = dram.tile([N, 520], BF)
        vB_g = dram.tile([N, 520], BF)
        den_d = dram.tile([16, R], F32)
        den_rd = dram.tile([16, R], BF)

        cst = ctx.enter_context(tc.tile_pool(name="cst", bufs=1))

        # warm-up collective at t~0: absorbs the ncfw barrier/setup latency.
        # Emitted first so its trigger is the first gpsimd/vector work.
        warm_s = cst.tile([1, 128], BF)
        with tc.high_priority():
            nc.vector.memset(warm_s, 1.0)
            nc.gpsimd.dma_start(out=warm_in, in_=warm_s)
            nc.gpsimd.collective_compute(
                "AllGather", mybir.AluOpType.bypass,
                ins=[warm_in.opt()], outs=[warm_out.opt()],
                replica_groups=GROUPS)

        atn = ctx.enter_context(tc.tile_pool(name="atn", bufs=1))
        xT_s = cst.tile([128, 8, R], BF)
        nc.sync.dma_start(out=xT_s, in_=xT[:, :].rearrange("(kt p) r -> p kt r", p=128))
        # k/q weight slots are recycled for the gathered-V tiles ("big" tag):
        # wk -> buf0, wq -> buf1, then vA reuses buf0 (after P1), vB buf1
        wkT_s = atn.tile([128, 8, C], BF, tag="big", bufs=2, name="wkT_s")
        # load the ct 0-3 half first so the K1-gather chain starts sooner
        _wk = wkT[:, :].rearrange("(kt p) c -> p kt c", p=128)
        for hh in range(2):
            nc.sync.dma_start(
                out=wkT_s[:, :, hh * 512:(hh + 1) * 512],
                in_=bass.AP(tensor=_wk.tensor, offset=_wk.offset + hh * 512,
                            ap=[list(_wk.ap[0]), list(_wk.ap[1]), [1, 512]]))
        wvT_s = cst.tile([128, 8, C], BF)
        nc.sync.dma_start(out=wvT_s, in_=wvT[:, :].rearrange("(kt p) c -> p kt c", p=128))
        wqT_s = atn.tile([128, 8, C], BF, tag="big", bufs=2, name="wqT_s")
        nc.sync.dma_start(out=wqT_s, in_=wqT[:, :].rearrange("(kt p) c -> p kt c", p=128))
        projT_s = cst.tile([128, 8, C], BF)
        nc.sync.dma_start(out=projT_s, in_=projT[:, :].rearrange("(kt p) c -> p kt c", p=128))
        biasT_s = cst.tile([1, C], BF)
        nc.sync.dma_start(out=biasT_s, in_=biasT[:, :])

        ones_s = cst.tile([1, R], BF)
        nc.vector.memset(ones_s, 1.0)
        kT_ls = cst.tile([128, 8, R], BF)
        qT_s = cst.tile([128, 8, R], BF)
        v_ls = cst.tile([128, 4, 1040], BF)
        nc.vector.memset(v_ls, 1.0)
        att_un = cst.tile([128, 8, R], BF)
        f6_s = cst.tile([128, 8, R], BF)
        att_s = cst.tile([128, 8, R], BF)
        rb_s = cst.tile([128, 8, R], BF)
        ps = ctx.enter_context(tc.tile_pool(name="ps", bufs=1, space="PSUM"))

        # ---- P1a: k columns 0..511 (heads 0-7), trigger K1 gather
        def k_block(ct):
            k_ps = ps.tile([128, R], F32, tag="mm", bufs=2, name=f"k_{ct}")
            for kt in range(8):
                nc.tensor.matmul(k_ps, wkT_s[:, kt, ct * 128:(ct + 1) * 128],
                                 xT_s[:, kt, :], start=(kt == 0), stop=(kt == 7))
            nc.vector.tensor_copy(kT_ls[:, ct, :], k_ps)

        def v_block(vc, rt):
            v_ps = ps.tile([128, R], F32, tag="mm", bufs=2, name=f"v_{vc}_{rt}")
            for kt in range(8):
                nc.tensor.matmul(v_ps, xT_s[:, kt, rt * 128:(rt + 1) * 128],
                                 wvT_s[:, kt, vc * 512:(vc + 1) * 512],
                                 start=(kt == 0), stop=(kt == 7))
            dst = v_ls[:, rt, vc * 520:(vc + 1) * 520]
            nc.vector.tensor_copy(_ap(dst, [[65, 8], [1, 64]]),
                                  v_ps[:, :].rearrange("p (h e) -> p h e", e=64))

        for ct in range(4):
            k_block(ct)
        with tc.high_priority():
            nc.gpsimd.dma_start(
                out=kA_b[:, :].rearrange("(ct p) r -> p ct r", p=128),
                in_=kT_ls[:, 0:4, :])
            nc.gpsimd.collective_compute(
                "AllGather", mybir.AluOpType.bypass,
                ins=[kA_b.opt()], outs=[kA_g.opt()], replica_groups=GROUPS)

        # ---- P2a: v columns 0..511 (heads 0-7), trigger V1 gather
        for rt in range(4):
            v_block(0, rt)
        with tc.high_priority():
            nc.gpsimd.dma_start(
                out=vA_b[:, :].rearrange("(rt p) c -> p rt c", p=128),
                in_=v_ls[:, :, 0:520])
            nc.gpsimd.collective_compute(
                "AllGather", mybir.AluOpType.bypass,
                ins=[vA_b.opt()], outs=[vA_g.opt()], replica_groups=GROUPS)

        # ---- P1b: k columns 512..1023 (heads 8-15), trigger K2
        for ct in range(4, 8):
            k_block(ct)
        with tc.high_priority():
            nc.gpsimd.dma_start(
                out=kB_b[:, :].rearrange("(ct p) r -> p ct r", p=128),
                in_=kT_ls[:, 4:8, :])
            nc.gpsimd.collective_compute(
                "AllGather", mybir.AluOpType.bypass,
                ins=[kB_b.opt()], outs=[kB_g.opt()], replica_groups=GROUPS)

        # ---- P2b: v columns 512..1023 (heads 8-15), trigger V2
        for rt in range(4):
            v_block(1, rt)
        with tc.high_priority():
            nc.gpsimd.dma_start(
                out=vB_b[:, :].rearrange("(rt p) c -> p rt c", p=128),
                in_=v_ls[:, :, 520:1040])
            nc.gpsimd.collective_compute(
                "AllGather", mybir.AluOpType.bypass,
                ins=[vB_b.opt()], outs=[vB_g.opt()], replica_groups=GROUPS)

        # ---- P3: q
        for ct in range(8):
            q_ps = ps.tile([128, R], F32, tag="mm", bufs=2, name=f"q_{ct}")
            for kt in range(8):
                nc.tensor.matmul(q_ps, wqT_s[:, kt, ct * 128:(ct + 1) * 128],
                                 xT_s[:, kt, :], start=(kt == 0), stop=(kt == 7))
            nc.vector.tensor_copy(qT_s[:, ct, :], q_ps)

        # gathered V -> SBUF in kt chunks so attn@V can start on the first
        # chunk while the rest stages (on gpsimd queue, behind the collectives)
        vA_s = atn.tile([128, KT, 520], BF, tag="big", bufs=2, name="vA_s")
        vB_s = atn.tile([128, KT, 520], BF, tag="big", bufs=2, name="vB_s")
        for vg, vsb in ((vA_g, vA_s), (vB_g, vB_s)):
            for q4 in range(4):
                nc.gpsimd.dma_start(
                    out=vsb[:, q4 * 4:(q4 + 1) * 4, :],
                    in_=vg[q4 * 512:(q4 + 1) * 512, :].rearrange(
                        "(kt p) c -> p kt c", p=128))

        # normalize a contiguous batch of head pairs: one reciprocal for the
        # batch, two partition-broadcast DMAs, one mul per head pair — all on
        # DVE/its DMA queue so nothing blocks the sync-queue kT_p prefetches
        def norm_batch(k0, n):
            den_l = atn.tile([2 * n, R], F32, tag=f"denl{k0}", bufs=1,
                             name=f"denl_{k0}")
            nc.gpsimd.dma_start(out=den_l, in_=den_d[2 * k0:2 * k0 + 2 * n, :])
            den_r = atn.tile([2 * n, R], BF, tag=f"denr{k0}", bufs=1,
                             name=f"denr_{k0}")
            with nc.allow_low_precision(reason="softmax denom recip to bf16"):
                nc.vector.reciprocal(den_r, den_l)
            nc.gpsimd.dma_start(out=den_rd[2 * k0:2 * k0 + 2 * n, :], in_=den_r)
            dr = den_rd[:, :]
            for j in range(2):
                nc.gpsimd.dma_start(
                    out=rb_s[j * 64:(j + 1) * 64, k0:k0 + n, :],
                    in_=bass.AP(tensor=dr.tensor,
                                offset=dr.offset + (2 * k0 + j) * R,
                                ap=[[0, 64], [2 * R, n], [1, R]]))
            for kp in range(k0, k0 + n):
                nc.vector.tensor_mul(att_s[:, kp, :], att_un[:, kp, :],
                                     rb_s[:, kp, :])

        def f6_ops():
            # kp0-5 projection partial, emitted op-by-op so it interleaves
            # into kp6/kp7's per-kt PE slack instead of blocking the queue
            for ct in range(8):
                f6_ps = ps.tile([128, R], F32, tag="mm", bufs=2,
                                name=f"f6_{ct}")
                for k6 in range(6):
                    nc.tensor.matmul(
                        f6_ps, projT_s[:, k6, ct * 128:(ct + 1) * 128],
                        att_s[:, k6, :], start=(k6 == 0), stop=(k6 == 5))
                    yield
                nc.vector.tensor_copy(f6_s[:, ct, :], f6_ps)
                yield
        f6_iter = [None]

        # ---- P4: attention, per head pair
        for kp in range(8):
            kg = (kA_g if kp < 4 else kB_g)[:, :]
            kpo = kp % 4
            vs_ = vA_s if kp < 4 else vB_s
            kT_p = atn.tile([128, 4, R], BF, tag="ktp", bufs=2, name=f"ktp_{kp}")
            nc.sync.dma_start(
                out=kT_p,
                in_=bass.AP(tensor=kg.tensor,
                            offset=kg.offset + kpo * 128 * R,
                            ap=[[R, 128], [4 * 128 * R, 4], [1, R]]))
            ao = [ps.tile([65, R], F32, tag=f"ao{j}", bufs=1, name=f"ao_{kp}_{j}")
                  for j in range(2)]
            # software pipeline: attn@V for tile kt-1 is emitted after the
            # scores+exp for kt, so the PE works while ScalarE runs exp
            def av(kt, ex):
                for j in range(2):
                    hj = 2 * kpo + j
                    nc.tensor.matmul(ao[j], vs_[:, kt, hj * 65:(hj + 1) * 65],
                                     ex[:, j, :],
                                     start=(kt == 0), stop=(kt == KT - 1))
            prev_ex = None
            for kt in range(KT):
                sp = ps.tile([128, 2, R], F32, tag="sp", bufs=2,
                             name=f"sp_{kp}_{kt}")
                for j in range(2):
                    nc.tensor.matmul(
                        sp[:, j, :],
                        kT_p[j * 64:(j + 1) * 64, kt // 4, (kt % 4) * 128:(kt % 4) * 128 + 128],
                        qT_s[j * 64:(j + 1) * 64, kp, :],
                        start=True, stop=True)
                ex = atn.tile([128, 2, R], BF, tag="exps", bufs=16,
                              name=f"ex_{kp}_{kt}")
                nc.scalar.activation(ex, sp, mybir.ActivationFunctionType.Exp,
                                     scale=0.125)
                if kt > 0:
                    av(kt - 1, prev_ex)
                prev_ex = ex
                if f6_iter[0] is not None:
                    next(f6_iter[0], None)
                    next(f6_iter[0], None)
            av(KT - 1, prev_ex)
            # drain denominators + unnormalized numerators
            for j in range(2):
                dstg = atn.tile([65, R], F32, tag="dstg", bufs=2,
                                name=f"dstg_{kp}_{j}")
                nc.vector.tensor_copy(dstg[64:65, :], ao[j][64:65, :])
                nc.gpsimd.dma_start(out=den_d[2 * kp + j:2 * kp + j + 1, :],
                                    in_=dstg[64:65, :])
                if j == 0:
                    nc.vector.tensor_copy(att_un[0:64, kp, :], ao[j][0:64, :])
                else:
                    tmp = atn.tile([64, R], F32, tag="tmpj", bufs=2,
                                   name=f"tmpj_{kp}")
                    nc.vector.tensor_copy(tmp, ao[j][0:64, :])
                    nc.gpsimd.dma_start(out=att_un[64:128, kp, :], in_=tmp)
            if kp == 5:
                norm_batch(0, 6)   # normalize kp 0-5 while kp 6-7 compute
                f6_iter[0] = f6_ops()
        if f6_iter[0] is not None:
            for _ in f6_iter[0]:
                pass
        norm_batch(6, 2)

        # ---- P5: remaining projection (kp 6-7) + bias + kp0-5 partial
        for ct in range(8):
            f_ps = ps.tile([128, R], F32, tag="mm", bufs=2, name=f"f_{ct}")
            for kp in (6, 7):
                nc.tensor.matmul(f_ps, projT_s[:, kp, ct * 128:(ct + 1) * 128],
                                 att_s[:, kp, :], start=(kp == 6), stop=False)
            nc.tensor.matmul(f_ps, biasT_s[:, ct * 128:(ct + 1) * 128], ones_s,
                             start=False, stop=True)
            f_s = atn.tile([128, R], F32, tag="fs", bufs=2, name=f"fs_{ct}")
            nc.vector.tensor_add(f_s, f_ps, f6_s[:, ct, :])
            nc.gpsimd.dma_start(out=outT[ct * 128:(ct + 1) * 128, :], in_=f_s)

        # consume the warm-up gather so its DMA completes inside the NEFF
        warm_back = cst.tile([4, 128], BF)
        nc.sync.dma_start(out=warm_back, in_=warm_out[:, :])
    _split_multi_waits(nc)
    return nc


def _split_multi_waits(nc):
    """This container's walrus supports one sync-wait per instruction; move
    extra waits onto preceding same-engine NoOps."""
    n_new = 0
    for bb in nc.m.functions[0].blocks:
        new = []
        for ins in bb.instructions:
            si = getattr(ins, "sync_info", None)
            ow = list(si.on_wait) if si is not None and si.on_wait else []
            if len(ow) > 1:
                for w in ow[:-1]:
                    n_new += 1
                    nop = mybir.InstNoOp(
                        name=f"{ins.name}_sw{n_new}",
                        engine=ins.engine,
                        sync_info=mybir.SyncInfo(on_wait=[w], on_update=[]),
                    )
                    new.append(nop)
                ins.sync_info = mybir.SyncInfo(
                    on_wait=[ow[-1]],
                    on_update=list(si.on_update) if si.on_update else [],
                )
            new.append(ins)
        bb.instructions = new


_NC = None
_LAST = None


def _ensure_ntff_hook():
    """The agent image's antenv lacks axon_hooks; shim it and register the
    ctypes NTFF profiler from trn_boot so trace=True yields exec_time_ns."""
    import sys
    import types
    try:
        import antenv.axon_hooks  # noqa: F401
        return
    except ImportError:
        pass
    mod = types.ModuleType("antenv.axon_hooks")
    holder = [None]
    mod.set_axon_ntff_profile_hook = lambda h: holder.__setitem__(0, h)
    mod.get_axon_ntff_profile_hook = lambda: holder[0]
    sys.modules["antenv.axon_hooks"] = mod
    import antenv
    antenv.axon_hooks = mod
    try:
        sys.path.insert(0, "/root/.axon_site")
        from trn_agent_boot.trn_boot import _ntff_profile_via_ctypes
        mod.set_axon_ntff_profile_hook(
            _ntff_profile_via_ctypes("/opt/axon/libaxon_pjrt.so"))
    except Exception:
        pass


def kernel(**inputs):
    global _NC, _LAST
    bf = ml_dtypes.bfloat16
    x = np.asarray(inputs["x"], np.float32)
    qkv_w = np.asarray(inputs["qkv_w"], np.float32)
    proj_w = np.asarray(inputs["proj_w"], np.float32)
    proj_b = np.asarray(inputs["proj_b"], np.float32)
    a1 = np.asarray(inputs["lora_w1_l1"], np.float32)
    b1 = np.asarray(inputs["lora_w1_l2"], np.float32)
    a2 = np.asarray(inputs["lora_w2_l1"], np.float32)
    b2 = np.asarray(inputs["lora_w2_l2"], np.float32)

    w_eff = qkv_w + 2.0 * (b1 @ a1)
    p_eff = proj_w + 2.0 * (b2 @ a2)
    shared = {
        "wqT": np.ascontiguousarray(w_eff[0:C].T).astype(bf),
        "wkT": np.ascontiguousarray(w_eff[C:2 * C].T).astype(bf),
        "wvT": np.ascontiguousarray(w_eff[2 * C:3 * C].T).astype(bf),
        "projT": np.ascontiguousarray(p_eff.T).astype(bf),
        "biasT": np.ascontiguousarray(proj_b[None, :]).astype(bf),
    }
    in_maps = []
    for c in range(8):
        g, r = divmod(c, 4)
        m = dict(shared)
        m["xT"] = np.ascontiguousarray(x[g, r * R:(r + 1) * R, :].T).astype(bf)
        in_maps.append(m)

    if _NC is None:
        _NC = build()
    trace = os.environ.get("ATT_TRACE", "0") == "1"
    if trace:
        _ensure_ntff_hook()
    _LAST = run_bass_kernel_spmd(_NC, in_maps, core_ids=list(range(8)),
                                 trace=trace)
    out = np.empty((B, N, C), np.float32)
    for c in range(8):
        g, r = divmod(c, 4)
        out[g, r * R:(r + 1) * R, :] = np.asarray(
            _LAST.results[c]["outT"], np.float32).T
    return out



# revision 10
# speedup vs baseline: 1.0417x; 1.0417x over previous
"""Trainium2 Bass kernel: 16-head attention with LoRA (B=2, N=2048, C=1024).

Sharding v2: batch x head-quad. Core c handles batch c//4 and heads
4*(c%4)..4*(c%4)+3 over the FULL 2048-token sequence, so Q/K/V and the
whole softmax need no collectives at all. The output projection is
computed as a partial product over the core's 256 attention dims and
combined with one ReduceScatter per 512-row block (groups [[0-3],[4-7]]),
which pipelines under the next block's attention. LoRA is folded into the
weights on the host. Scores are computed transposed (keys on partitions),
exp on ScalarE (the bottleneck engine, ~1.15us per 128x1024 tile), with
the attn@V numerator+denominator fused via a ones-column (M=65 trick).
Background PE work (remaining V tiles, Q tiles, proj partials) is dripped
into the PE slack between attention matmuls.
"""

import os
from collections import deque
from contextlib import ExitStack

import numpy as np
import ml_dtypes

import concourse.bass as bass
import concourse.mybir as mybir
import concourse.tile as tile
from concourse.bass_utils import run_bass_kernel_spmd

B, N, C, H, D = 2, 2048, 1024, 16, 64
RC = 512         # query rows per chunk / row block
RB = 4           # row blocks
KC = 16          # key chunks of 128
BF = mybir.dt.bfloat16
F32 = mybir.dt.float32
GROUPS = [[0, 1, 2, 3], [4, 5, 6, 7]]
QBLK = 131072    # 1024 out-dims x 128 rows: one ReduceScatter output quarter


def _ap(src, dims):
    """Rebuild an AP keeping its partition dim but with custom free dims."""
    return bass.AP(tensor=src.tensor, offset=src.offset,
                   ap=[list(src.ap[0])] + [list(d) for d in dims])


def build():
    nc = bass.Bass()
    xT = nc.declare_dram_parameter("xT", [C, N], BF, isOutput=False)
    wqT = nc.declare_dram_parameter("wqT", [C, 256], BF, isOutput=False)
    wkT = nc.declare_dram_parameter("wkT", [C, 256], BF, isOutput=False)
    wvT = nc.declare_dram_parameter("wvT", [C, 256], BF, isOutput=False)
    projT = nc.declare_dram_parameter("projT", [256, C], BF, isOutput=False)
    biasT = nc.declare_dram_parameter("biasT", [1, C], BF, isOutput=False)
    outT = nc.declare_dram_parameter("outT", [RB, QBLK], BF, isOutput=True)

    with tile.TileContext(nc) as tc, ExitStack() as ctx:
        dram = ctx.enter_context(tc.tile_pool(name="dram", bufs=1, space="DRAM"))
        warm_in = dram.tile([1, 128], BF)
        warm_out = dram.tile([4, 128], BF)
        stage = [dram.tile([4, QBLK], BF, name=f"stage_{i}") for i in range(RB)]
        rs_out = [dram.tile([1, QBLK], BF, name=f"rs_out_{i}") for i in range(RB)]
        rec_d = dram.tile([RB, 4 * RC], BF)

        cst = ctx.enter_context(tc.tile_pool(name="cst", bufs=1))

        # warm-up collective at t~0 absorbs the ncfw barrier (~34us).
        warm_s = cst.tile([1, 128], BF)
        with tc.high_priority():
            nc.vector.memset(warm_s, 1.0)
            nc.gpsimd.dma_start(out=warm_in, in_=warm_s)
            nc.gpsimd.collective_compute(
                "AllGather", mybir.AluOpType.bypass,
                ins=[warm_in.opt()], outs=[warm_out.opt()],
                replica_groups=GROUPS)

        # ---- input loads, split across the two DMA queues by first use
        xT_s = cst.tile([128, 8, N], BF)
        wk_s = cst.tile([128, 8, 256], BF)
        wq_s = cst.tile([128, 8, 256], BF)
        wv_s = cst.tile([128, 8, 256], BF)
        projT_s = cst.tile([128, 2, C], BF)
        biasT_s = cst.tile([1, C], BF)
        nc.sync.dma_start(out=wk_s, in_=wkT[:, :].rearrange("(kt p) d -> p kt d", p=128))
        for kt in (0, 2, 4, 6):
            nc.sync.dma_start(out=xT_s[:, kt, :], in_=xT[kt * 128:(kt + 1) * 128, :])
        nc.gpsimd.dma_start(out=wq_s, in_=wqT[:, :].rearrange("(kt p) d -> p kt d", p=128))
        for kt in (1, 3, 5, 7):
            nc.gpsimd.dma_start(out=xT_s[:, kt, :], in_=xT[kt * 128:(kt + 1) * 128, :])
        nc.sync.dma_start(out=wv_s, in_=wvT[:, :].rearrange("(kt p) d -> p kt d", p=128))
        nc.sync.dma_start(out=biasT_s, in_=biasT[:, :])
        nc.gpsimd.dma_start(out=projT_s, in_=projT[:, :].rearrange("(kt p) c -> p kt c", p=128))

        kT_s = cst.tile([128, 2, N], BF)
        qT_s = cst.tile([128, 2, N], BF)
        v_s = cst.tile([128, KC, 260], BF)   # 4 heads x (64 dims + ones col)
        nc.vector.memset(v_s, 1.0)
        ones_s = cst.tile([1, RC], BF)
        nc.vector.memset(ones_s, 1.0)

        atn = ctx.enter_context(tc.tile_pool(name="atn", bufs=1))
        ps = ctx.enter_context(tc.tile_pool(name="ps", bufs=1, space="PSUM"))

        def kq_block(w_s, dst, p, rc, nm):
            t = ps.tile([128, RC], F32, tag="mm", bufs=2, name=f"{nm}_{p}_{rc}")
            for kt in range(8):
                nc.tensor.matmul(t, w_s[:, kt, p * 128:(p + 1) * 128],
                                 xT_s[:, kt, rc * RC:(rc + 1) * RC],
                                 start=(kt == 0), stop=(kt == 7))
            nc.vector.tensor_copy(dst[:, p, rc * RC:(rc + 1) * RC], t)

        def v_block(kc):
            t = ps.tile([128, RC], F32, tag="mm", bufs=2, name=f"v_{kc}")
            for kt in range(8):
                nc.tensor.matmul(t[:, 0:256], xT_s[:, kt, kc * 128:(kc + 1) * 128],
                                 wv_s[:, kt, :], start=(kt == 0), stop=(kt == 7))
            nc.vector.tensor_copy(
                _ap(v_s[:, kc, :], [[65, 4], [1, 64]]),
                t[:, 0:256].rearrange("p (h e) -> p h e", e=64))

        # ---- upfront PE work: all of kT, qT for row block 0, v kc 0-5
        for p in range(2):
            for rc in range(4):
                kq_block(wk_s, kT_s, p, rc, "k")
        for p in range(2):
            kq_block(wq_s, qT_s, p, 0, "q")
        for kc in range(6):
            v_block(kc)

        # ---- background work dripped into attention PE slack
        def v_gen():
            for kc in range(6, KC):
                t = ps.tile([128, RC], F32, tag="mm", bufs=2, name=f"v_{kc}")
                for kt in range(8):
                    nc.tensor.matmul(t[:, 0:256],
                                     xT_s[:, kt, kc * 128:(kc + 1) * 128],
                                     wv_s[:, kt, :], start=(kt == 0), stop=(kt == 7))
                    yield
                nc.vector.tensor_copy(
                    _ap(v_s[:, kc, :], [[65, 4], [1, 64]]),
                    t[:, 0:256].rearrange("p (h e) -> p h e", e=64))
                yield

        def q_gen():
            for rc in range(1, 4):
                for p in range(2):
                    t = ps.tile([128, RC], F32, tag="mm", bufs=2, name=f"q_{p}_{rc}")
                    for kt in range(8):
                        nc.tensor.matmul(t, wq_s[:, kt, p * 128:(p + 1) * 128],
                                         xT_s[:, kt, rc * RC:(rc + 1) * RC],
                                         start=(kt == 0), stop=(kt == 7))
                        yield
                    nc.vector.tensor_copy(qT_s[:, p, rc * RC:(rc + 1) * RC], t)
                    yield

        def proj_gen(rb, att_rb):
            po_s = atn.tile([128, 8, RC], BF, tag="po", bufs=2, name=f"po_{rb}")
            for ct in range(8):
                t = ps.tile([128, RC], F32, tag="mm", bufs=2, name=f"f_{rb}_{ct}")
                nc.tensor.matmul(t, projT_s[:, 0, ct * 128:(ct + 1) * 128],
                                 att_rb[:, 0, :], start=True, stop=False)
                yield
                nc.tensor.matmul(t, projT_s[:, 1, ct * 128:(ct + 1) * 128],
                                 att_rb[:, 1, :], start=False, stop=False)
                yield
                nc.tensor.matmul(t, biasT_s[:, ct * 128:(ct + 1) * 128], ones_s,
                                 start=False, stop=True)
                yield
                nc.vector.tensor_copy(po_s[:, ct, :], t)
                yield
            # stage [od, r] -> [quarter, od, row-in-quarter], then ReduceScatter
            stg = stage[rb][:, :]
            for ct in range(8):
                nc.gpsimd.dma_start(
                    out=bass.AP(tensor=stg.tensor,
                                offset=stg.offset + ct * 128 * 128,
                                ap=[[128, 128], [QBLK, 4], [1, 128]]),
                    in_=_ap(po_s[:, ct, :], [[128, 4], [1, 128]]))
                yield
            nc.gpsimd.collective_compute(
                "ReduceScatter", mybir.AluOpType.add,
                ins=[stage[rb].opt()], outs=[rs_out[rb].opt()],
                replica_groups=GROUPS)
            yield
            nc.gpsimd.dma_start(out=outT[rb:rb + 1, :], in_=rs_out[rb][:, :])
            yield

        bg = deque([v_gen(), q_gen()])
        _DONE = object()

        def drip(n):
            while n > 0 and bg:
                if next(bg[0], _DONE) is _DONE:
                    bg.popleft()
                else:
                    n -= 1

        # ---- attention: per (row block, head pair): scores -> exp -> attn@V
        for rb in range(RB):
            att_un = atn.tile([128, 2, RC], BF, tag="attu", bufs=2, name=f"attu_{rb}")
            den_l = atn.tile([4, RC], F32, tag="den", bufs=2, name=f"den_{rb}")
            for p in range(2):
                ao = ps.tile([65, 2, RC], F32, tag="ao", bufs=1, name=f"ao_{rb}_{p}")

                def av(kc, ex_t):
                    for j in range(2):
                        h = 2 * p + j
                        nc.tensor.matmul(ao[:, j, :], v_s[:, kc, h * 65:(h + 1) * 65],
                                         ex_t[:, j, :],
                                         start=(kc == 0), stop=(kc == KC - 1))

                prev_ex = None
                for kc in range(KC):
                    sp = ps.tile([128, 2, RC], F32, tag="sp", bufs=2,
                                 name=f"sp_{rb}_{p}_{kc}")
                    for j in range(2):
                        nc.tensor.matmul(
                            sp[:, j, :],
                            kT_s[64 * j:64 * j + 64, p, kc * 128:(kc + 1) * 128],
                            qT_s[64 * j:64 * j + 64, p, rb * RC:(rb + 1) * RC],
                            start=True, stop=True)
                    ex_t = atn.tile([128, 2, RC], BF, tag="ex", bufs=12,
                                    name=f"ex_{rb}_{p}_{kc}")
                    nc.scalar.activation(ex_t, sp, mybir.ActivationFunctionType.Exp,
                                         scale=0.125)
                    if kc > 0:
                        av(kc - 1, prev_ex)
                    prev_ex = ex_t
                    drip(6 if (rb == 0 and p == 0) else 2)
                av(KC - 1, prev_ex)
                # drain this pair: unnormalized numerators + denominators.
                # DVE can't cross partitions, so j=1 bounces through a DMA.
                nc.vector.tensor_copy(att_un[0:64, p, :], ao[0:64, 0, :])
                tmp = atn.tile([64, RC], BF, tag="tmpj", bufs=2,
                               name=f"tmpj_{rb}_{p}")
                nc.vector.tensor_copy(tmp, ao[0:64, 1, :])
                nc.gpsimd.dma_start(out=att_un[64:128, p, :], in_=tmp)
                dstg = atn.tile([65, 2 * RC], F32, tag="dstg", bufs=2,
                                name=f"dstg_{rb}_{p}")
                nc.vector.tensor_copy(
                    _ap(dstg[64:65, :], [[RC, 2], [1, RC]]), ao[64:65, :, :])
                nc.gpsimd.dma_start(out=den_l[2 * p:2 * p + 2, :],
                                    in_=dstg[64:65, :])
            # normalize the row block: one reciprocal over 4 (pair,j) rows,
            # broadcast via DRAM bounce, then two muls
            den_r = atn.tile([4, RC], BF, tag="denr", bufs=2, name=f"denr_{rb}")
            with nc.allow_low_precision(reason="softmax denom recip to bf16"):
                nc.vector.reciprocal(den_r, den_l)
            rd = rec_d[rb:rb + 1, :]
            nc.gpsimd.dma_start(out=rd, in_=den_r)
            rb_s = atn.tile([128, 2, RC], BF, tag="rbs", bufs=2, name=f"rbs_{rb}")
            for j in range(2):
                nc.gpsimd.dma_start(
                    out=rb_s[64 * j:64 * j + 64, :, :],
                    in_=bass.AP(tensor=rd.tensor, offset=rd.offset + j * RC,
                                ap=[[0, 64], [2 * RC, 2], [1, RC]]))
            att_rb = atn.tile([128, 2, RC], BF, tag="att", bufs=2, name=f"att_{rb}")
            for p in range(2):
                nc.vector.tensor_mul(att_rb[:, p, :], att_un[:, p, :], rb_s[:, p, :])
            bg.append(proj_gen(rb, att_rb))

        # flush remaining background work (proj of the last row blocks)
        while bg:
            if next(bg[0], _DONE) is _DONE:
                bg.popleft()

        # consume the warm-up gather so its DMA completes inside the NEFF
        warm_back = cst.tile([4, 128], BF)
        nc.sync.dma_start(out=warm_back, in_=warm_out[:, :])
    _split_multi_waits(nc)
    return nc


def _split_multi_waits(nc):
    """This container's walrus supports one sync-wait per instruction; move
    extra waits onto preceding same-engine NoOps."""
    n_new = 0
    for bb in nc.m.functions[0].blocks:
        new = []
        for ins in bb.instructions:
            si = getattr(ins, "sync_info", None)
            ow = list(si.on_wait) if si is not None and si.on_wait else []
            if len(ow) > 1:
                for w in ow[:-1]:
                    n_new += 1
                    nop = mybir.InstNoOp(
                        name=f"{ins.name}_sw{n_new}",
                        engine=ins.engine,
                        sync_info=mybir.SyncInfo(on_wait=[w], on_update=[]),
                    )
                    new.append(nop)
                ins.sync_info = mybir.SyncInfo(
                    on_wait=[ow[-1]],
                    on_update=list(si.on_update) if si.on_update else [],
                )
            new.append(ins)
        bb.instructions = new


_NC = None
_LAST = None


def _ensure_ntff_hook():
    """The agent image's antenv lacks axon_hooks; shim it and register the
    ctypes NTFF profiler from trn_boot so trace=True yields exec_time_ns."""
    import sys
    import types
    try:
        import antenv.axon_hooks  # noqa: F401
        return
    except ImportError:
        pass
    mod = types.ModuleType("antenv.axon_hooks")
    holder = [None]
    mod.set_axon_ntff_profile_hook = lambda h: holder.__setitem__(0, h)
    mod.get_axon_ntff_profile_hook = lambda: holder[0]
    sys.modules["antenv.axon_hooks"] = mod
    import antenv
    antenv.axon_hooks = mod
    try:
        sys.path.insert(0, "/root/.axon_site")
        from trn_agent_boot.trn_boot import _ntff_profile_via_ctypes
        mod.set_axon_ntff_profile_hook(
            _ntff_profile_via_ctypes("/opt/axon/libaxon_pjrt.so"))
    except Exception:
        pass


def kernel(**inputs):
    global _NC, _LAST
    bf = ml_dtypes.bfloat16
    x = np.asarray(inputs["x"], np.float32)
    qkv_w = np.asarray(inputs["qkv_w"], np.float32)
    proj_w = np.asarray(inputs["proj_w"], np.float32)
    proj_b = np.asarray(inputs["proj_b"], np.float32)
    a1 = np.asarray(inputs["lora_w1_l1"], np.float32)
    b1 = np.asarray(inputs["lora_w1_l2"], np.float32)
    a2 = np.asarray(inputs["lora_w2_l1"], np.float32)
    b2 = np.asarray(inputs["lora_w2_l2"], np.float32)

    w_eff = qkv_w + 2.0 * (b1 @ a1)
    p_eff = proj_w + 2.0 * (b2 @ a2)
    zeros_bias = np.zeros_like(proj_b)
    in_maps = []
    for c in range(8):
        g, q = divmod(c, 4)
        ds = slice(256 * q, 256 * q + 256)
        m = {
            "xT": np.ascontiguousarray(x[g].T).astype(bf),
            "wqT": np.ascontiguousarray(w_eff[0:C][ds].T).astype(bf),
            "wkT": np.ascontiguousarray(w_eff[C:2 * C][ds].T).astype(bf),
            "wvT": np.ascontiguousarray(w_eff[2 * C:3 * C][ds].T).astype(bf),
            "projT": np.ascontiguousarray(p_eff[:, ds].T).astype(bf),
            "biasT": np.ascontiguousarray(
                (proj_b if q == 0 else zeros_bias)[None, :]).astype(bf),
        }
        in_maps.append(m)

    if _NC is None:
        _NC = build()
    trace = os.environ.get("ATT_TRACE", "0") == "1"
    if trace:
        _ensure_ntff_hook()
    _LAST = run_bass_kernel_spmd(_NC, in_maps, core_ids=list(range(8)),
                                 trace=trace)
    out = np.empty((B, N, C), np.float32)
    for c in range(8):
        g, i = divmod(c, 4)
        res = np.asarray(_LAST.results[c]["outT"], np.float32).reshape(RB, C, 128)
        for rb in range(RB):
            out[g, rb * RC + i * 128: rb * RC + (i + 1) * 128, :] = res[rb].T
    return out


# revision 11
# speedup vs baseline: 1.4169x; 1.3601x over previous
"""Trainium2 Bass kernel: 16-head attention with LoRA (B=2, N=2048, C=1024).

Sharding v3: batch x head-quad, zero collectives. Core c handles batch
c//4 and heads 4*(c%4)..4*(c%4)+3 over the full 2048-token sequence, so
Q/K/V and the softmax need no cross-core communication. The output
projection is computed as a per-core PARTIAL product over the core's 256
attention dims and written out in f32; the HOST sums the 4 partials per
batch and adds the bias (part of unsharding). This removes the collective
barrier (~34us), the slow ReduceScatter ops, and their queue serialization.

Attention: scores transposed (keys on partitions), pairs of heads packed
as row-tiles (K=64 x 2), exp on ScalarE (the floor: ~147us/core), attn@V
packed as col-tiles (M=64 x 2, tile_position), softmax denominators from
a DVE-accumulated sum of exp tiles + one ones-vector matmul per head.
Background PE work (V tiles, Q tiles, proj partials) drips into the PE
slack between attention matmuls.
"""

import os
from collections import deque
from contextlib import ExitStack

import numpy as np
import ml_dtypes

import concourse.bass as bass
import concourse.mybir as mybir
import concourse.tile as tile
from concourse.bass_utils import run_bass_kernel_spmd

B, N, C, H, D = 2, 2048, 1024, 16, 64
RC = 512         # query rows per chunk / row block
RB = 4           # row blocks
KC = 16          # key chunks of 128
BF = mybir.dt.bfloat16
F32 = mybir.dt.float32
OBLK = C * RC    # one row block of partial output: [1024 od, 512 r]


def _ap(src, dims):
    """Rebuild an AP keeping its partition dim but with custom free dims."""
    return bass.AP(tensor=src.tensor, offset=src.offset,
                   ap=[list(src.ap[0])] + [list(d) for d in dims])


def build():
    nc = bass.Bass()
    xT = nc.declare_dram_parameter("xT", [C, N], BF, isOutput=False)
    wqT = nc.declare_dram_parameter("wqT", [C, 256], BF, isOutput=False)
    wkT = nc.declare_dram_parameter("wkT", [C, 256], BF, isOutput=False)
    wvT = nc.declare_dram_parameter("wvT", [C, 256], BF, isOutput=False)
    projT = nc.declare_dram_parameter("projT", [256, C], BF, isOutput=False)
    outT = nc.declare_dram_parameter("outT", [RB, OBLK], F32, isOutput=True)

    with tile.TileContext(nc) as tc, ExitStack() as ctx:
        dram = ctx.enter_context(tc.tile_pool(name="dram", bufs=1, space="DRAM"))
        rec_d = dram.tile([RB, 4 * RC], BF)

        cst = ctx.enter_context(tc.tile_pool(name="cst", bufs=1))

        # ---- input loads, split across the two DMA queues by first use
        xT_s = cst.tile([128, 8, N], BF)
        wk_s = cst.tile([128, 8, 256], BF)
        wq_s = cst.tile([128, 8, 256], BF)
        wv_s = cst.tile([128, 8, 256], BF)
        projT_s = cst.tile([128, 2, C], BF)
        nc.sync.dma_start(out=wk_s, in_=wkT[:, :].rearrange("(kt p) d -> p kt d", p=128))
        for kt in (0, 2, 4, 6):
            nc.sync.dma_start(out=xT_s[:, kt, :], in_=xT[kt * 128:(kt + 1) * 128, :])
        nc.gpsimd.dma_start(out=wq_s, in_=wqT[:, :].rearrange("(kt p) d -> p kt d", p=128))
        for kt in (1, 3, 5, 7):
            nc.gpsimd.dma_start(out=xT_s[:, kt, :], in_=xT[kt * 128:(kt + 1) * 128, :])
        nc.sync.dma_start(out=wv_s, in_=wvT[:, :].rearrange("(kt p) d -> p kt d", p=128))
        nc.gpsimd.dma_start(out=projT_s, in_=projT[:, :].rearrange("(kt p) c -> p kt c", p=128))

        kT_s = cst.tile([128, 2, N], BF)
        qT_s = cst.tile([128, 2, N], BF)
        v_s = cst.tile([128, KC, 256], BF)
        ones_c = cst.tile([128, 1], BF)
        nc.vector.memset(ones_c, 1.0)

        atn = ctx.enter_context(tc.tile_pool(name="atn", bufs=1))
        ps = ctx.enter_context(tc.tile_pool(name="ps", bufs=1, space="PSUM"))

        def kq_block(w_s, dst, p, rc, nm):
            t = ps.tile([128, RC], F32, tag="mm", bufs=2, name=f"{nm}_{p}_{rc}")
            for kt in range(8):
                nc.tensor.matmul(t, w_s[:, kt, p * 128:(p + 1) * 128],
                                 xT_s[:, kt, rc * RC:(rc + 1) * RC],
                                 start=(kt == 0), stop=(kt == 7))
            nc.vector.tensor_copy(dst[:, p, rc * RC:(rc + 1) * RC], t)

        def v_block(kc):
            t = ps.tile([128, RC], F32, tag="mm", bufs=2, name=f"v_{kc}")
            for kt in range(8):
                nc.tensor.matmul(t[:, 0:256], xT_s[:, kt, kc * 128:(kc + 1) * 128],
                                 wv_s[:, kt, :], start=(kt == 0), stop=(kt == 7))
            nc.vector.tensor_copy(v_s[:, kc, :], t[:, 0:256])

        # ---- upfront PE work: all of kT, qT for row block 0, v kc 0-5
        for p in range(2):
            for rc in range(4):
                kq_block(wk_s, kT_s, p, rc, "k")
        for p in range(2):
            kq_block(wq_s, qT_s, p, 0, "q")
        for kc in range(6):
            v_block(kc)

        # ---- background work dripped into attention PE slack
        def v_gen():
            for kc in range(6, KC):
                t = ps.tile([128, RC], F32, tag="mm", bufs=2, name=f"v_{kc}")
                for kt in range(8):
                    nc.tensor.matmul(t[:, 0:256],
                                     xT_s[:, kt, kc * 128:(kc + 1) * 128],
                                     wv_s[:, kt, :], start=(kt == 0), stop=(kt == 7))
                    yield
                nc.vector.tensor_copy(v_s[:, kc, :], t[:, 0:256])
                yield

        def q_gen():
            for rc in range(1, 4):
                for p in range(2):
                    t = ps.tile([128, RC], F32, tag="mm", bufs=2, name=f"q_{p}_{rc}")
                    for kt in range(8):
                        nc.tensor.matmul(t, wq_s[:, kt, p * 128:(p + 1) * 128],
                                         xT_s[:, kt, rc * RC:(rc + 1) * RC],
                                         start=(kt == 0), stop=(kt == 7))
                        yield
                    nc.vector.tensor_copy(qT_s[:, p, rc * RC:(rc + 1) * RC], t)
                    yield

        def proj_gen(rb, att_rb):
            po_s = atn.tile([128, 8, RC], F32, tag="po", bufs=2, name=f"po_{rb}")
            for ct in range(8):
                t = ps.tile([128, RC], F32, tag="mm", bufs=2, name=f"f_{rb}_{ct}")
                nc.tensor.matmul(t, projT_s[:, 0, ct * 128:(ct + 1) * 128],
                                 att_rb[:, 0, :], start=True, stop=False)
                yield
                nc.tensor.matmul(t, projT_s[:, 1, ct * 128:(ct + 1) * 128],
                                 att_rb[:, 1, :], start=False, stop=True)
                yield
                nc.vector.tensor_copy(po_s[:, ct, :], t)
                yield
            ot = outT[rb:rb + 1, :]
            nc.sync.dma_start(
                out=bass.AP(tensor=ot.tensor, offset=ot.offset,
                            ap=[[RC, 128], [128 * RC, 8], [1, RC]]),
                in_=po_s)
            yield

        bg = deque([v_gen(), q_gen()])
        _DONE = object()

        def drip(n):
            while n > 0 and bg:
                if next(bg[0], _DONE) is _DONE:
                    bg.popleft()
                else:
                    n -= 1

        # ---- attention: per (row block, head pair): scores -> exp -> attn@V
        for rb in range(RB):
            den_l = atn.tile([4, RC], F32, tag="den", bufs=2, name=f"den_{rb}")
            aos = []
            for p in range(2):
                ao = ps.tile([128, RC], F32, tag="ao", bufs=2, name=f"ao_{rb}_{p}")
                aos.append(ao)
                exs = atn.tile([128, 2, RC], BF, tag="exs", bufs=2,
                               name=f"exs_{rb}_{p}")

                def av(kc, ex_t):
                    for j in range(2):
                        nc.tensor.matmul(ao[64 * j:64 * j + 64, :],
                                         v_s[:, kc, (2 * p + j) * 64:(2 * p + j + 1) * 64],
                                         ex_t[:, j, :],
                                         start=(kc == 0), stop=(kc == KC - 1),
                                         tile_position=(0, 64 * j))

                prev_ex = None
                for kc in range(KC):
                    sp = ps.tile([128, 2, RC], F32, tag="sp", bufs=2,
                                 name=f"sp_{rb}_{p}_{kc}")
                    for j in range(2):
                        nc.tensor.matmul(
                            sp[:, j, :],
                            kT_s[64 * j:64 * j + 64, p, kc * 128:(kc + 1) * 128],
                            qT_s[64 * j:64 * j + 64, p, rb * RC:(rb + 1) * RC],
                            start=True, stop=True)
                    ex_t = atn.tile([128, 2, RC], BF, tag="ex", bufs=12,
                                    name=f"ex_{rb}_{p}_{kc}")
                    nc.scalar.activation(ex_t, sp, mybir.ActivationFunctionType.Exp,
                                         scale=0.125)
                    if kc == 0:
                        nc.vector.tensor_copy(exs, ex_t)
                    else:
                        nc.vector.tensor_add(exs, exs, ex_t)
                        av(kc - 1, prev_ex)
                    prev_ex = ex_t
                    drip(6 if (rb == 0 and p == 0) else 2)
                av(KC - 1, prev_ex)
                # denominators: ones.T @ exs -> [1, 512] per head, via PE
                for j in range(2):
                    dt_ = ps.tile([128, RC], F32, tag="mm", bufs=2,
                                  name=f"dn_{rb}_{p}_{j}")
                    nc.tensor.matmul(dt_[0:1, :], ones_c, exs[:, j, :],
                                     start=True, stop=True)
                    dsg = atn.tile([1, RC], F32, tag="dsg", bufs=4,
                                   name=f"dsg_{rb}_{p}_{j}")
                    nc.vector.tensor_copy(dsg, dt_[0:1, :])
                    nc.gpsimd.dma_start(out=den_l[2 * p + j:2 * p + j + 1, :],
                                        in_=dsg)
            # normalize the row block: reciprocal over the 4 (pair,j) rows,
            # broadcast via DRAM bounce, then one mul per pair from PSUM
            den_r = atn.tile([4, RC], BF, tag="denr", bufs=2, name=f"denr_{rb}")
            with nc.allow_low_precision(reason="softmax denom recip to bf16"):
                nc.vector.reciprocal(den_r, den_l)
            rd = rec_d[rb:rb + 1, :]
            nc.gpsimd.dma_start(out=rd, in_=den_r)
            rb_s = atn.tile([128, 2, RC], BF, tag="rbs", bufs=2, name=f"rbs_{rb}")
            for j in range(2):
                nc.gpsimd.dma_start(
                    out=rb_s[64 * j:64 * j + 64, :, :],
                    in_=bass.AP(tensor=rd.tensor, offset=rd.offset + j * RC,
                                ap=[[0, 64], [2 * RC, 2], [1, RC]]))
            att_rb = atn.tile([128, 2, RC], BF, tag="att", bufs=2, name=f"att_{rb}")
            for p in range(2):
                nc.vector.tensor_mul(att_rb[:, p, :], aos[p], rb_s[:, p, :])
            bg.append(proj_gen(rb, att_rb))

        # flush remaining background work (proj of the last row blocks)
        while bg:
            if next(bg[0], _DONE) is _DONE:
                bg.popleft()
    _split_multi_waits(nc)
    return nc


def _split_multi_waits(nc):
    """This container's walrus supports one sync-wait per instruction; move
    extra waits onto preceding same-engine NoOps."""
    n_new = 0
    for bb in nc.m.functions[0].blocks:
        new = []
        for ins in bb.instructions:
            si = getattr(ins, "sync_info", None)
            ow = list(si.on_wait) if si is not None and si.on_wait else []
            if len(ow) > 1:
                for w in ow[:-1]:
                    n_new += 1
                    nop = mybir.InstNoOp(
                        name=f"{ins.name}_sw{n_new}",
                        engine=ins.engine,
                        sync_info=mybir.SyncInfo(on_wait=[w], on_update=[]),
                    )
                    new.append(nop)
                ins.sync_info = mybir.SyncInfo(
                    on_wait=[ow[-1]],
                    on_update=list(si.on_update) if si.on_update else [],
                )
            new.append(ins)
        bb.instructions = new


_NC = None
_LAST = None


def _ensure_ntff_hook():
    """The agent image's antenv lacks axon_hooks; shim it and register the
    ctypes NTFF profiler from trn_boot so trace=True yields exec_time_ns."""
    import sys
    import types
    try:
        import antenv.axon_hooks  # noqa: F401
        return
    except ImportError:
        pass
    mod = types.ModuleType("antenv.axon_hooks")
    holder = [None]
    mod.set_axon_ntff_profile_hook = lambda h: holder.__setitem__(0, h)
    mod.get_axon_ntff_profile_hook = lambda: holder[0]
    sys.modules["antenv.axon_hooks"] = mod
    import antenv
    antenv.axon_hooks = mod
    try:
        sys.path.insert(0, "/root/.axon_site")
        from trn_agent_boot.trn_boot import _ntff_profile_via_ctypes
        mod.set_axon_ntff_profile_hook(
            _ntff_profile_via_ctypes("/opt/axon/libaxon_pjrt.so"))
    except Exception:
        pass


def kernel(**inputs):
    global _NC, _LAST
    bf = ml_dtypes.bfloat16
    x = np.asarray(inputs["x"], np.float32)
    qkv_w = np.asarray(inputs["qkv_w"], np.float32)
    proj_w = np.asarray(inputs["proj_w"], np.float32)
    proj_b = np.asarray(inputs["proj_b"], np.float32)
    a1 = np.asarray(inputs["lora_w1_l1"], np.float32)
    b1 = np.asarray(inputs["lora_w1_l2"], np.float32)
    a2 = np.asarray(inputs["lora_w2_l1"], np.float32)
    b2 = np.asarray(inputs["lora_w2_l2"], np.float32)

    w_eff = qkv_w + 2.0 * (b1 @ a1)
    p_eff = proj_w + 2.0 * (b2 @ a2)
    in_maps = []
    for c in range(8):
        g, q = divmod(c, 4)
        ds = slice(256 * q, 256 * q + 256)
        m = {
            "xT": np.ascontiguousarray(x[g].T).astype(bf),
            "wqT": np.ascontiguousarray(w_eff[0:C][ds].T).astype(bf),
            "wkT": np.ascontiguousarray(w_eff[C:2 * C][ds].T).astype(bf),
            "wvT": np.ascontiguousarray(w_eff[2 * C:3 * C][ds].T).astype(bf),
            "projT": np.ascontiguousarray(p_eff[:, ds].T).astype(bf),
        }
        in_maps.append(m)

    if _NC is None:
        _NC = build()
    trace = os.environ.get("ATT_TRACE", "0") == "1"
    if trace:
        _ensure_ntff_hook()
    _LAST = run_bass_kernel_spmd(_NC, in_maps, core_ids=list(range(8)),
                                 trace=trace)
    # host-side unshard: sum the 4 head-quad partials per batch, add bias,
    # transpose [od, r] -> [r, od]
    out = np.zeros((B, N, C), np.float32)
    for c in range(8):
        g = c // 4
        res = np.asarray(_LAST.results[c]["outT"], np.float32).reshape(RB, C, RC)
        for rb in range(RB):
            out[g, rb * RC:(rb + 1) * RC, :] += res[rb].T
    out += proj_b[None, None, :]
    return out
